# revision 1
# baseline (speedup 1.0000x reference)
"""Trainium2 Bass kernel for nn_Attention_update (additive attention pooling).

reference math (per example b):
    pre[s,d] = enc[b] @ W1e^T + (W1h @ h[b] + b1)      # [S, D]
    e[s]     = tanh(pre) @ W2[0]                        # [S]
    alpha    = softmax(e);  ctx = alpha @ enc[b]        # [DK]

Sharding: data-parallel over batch B=64 across 8 cores (8 examples/core),
same SPMD program on every core, no collectives.

Per-core kernel (score matmuls in float32r -> PE fast path, ~1 cyc/row,
~1e-4 matmul precision):
  - scores: per 128-row s-tile, PSUM chain over 8 k-chunks with stationary
    = transposed-enc tile [128k x 128s], moving = W1e^T [128k x 512d]
    -> pre in [s partitions, d free] layout.
  - bias row (W1h@h+b1, computed once at startup on PE) is broadcast to all
    128 partitions via a stride-0 DMA and added on VectorE while evacuating
    PSUM; ScalarE applies tanh.
  - e[s] = sum_d tanh*W2 in one fused VectorE scalar_tensor_tensor with
    accum_out -> e lands directly as [128, 16] (s on partitions), so the
    softmax needs no transposes.
  - p = exp(e) (no max subtraction: |e| is O(1) for this model); sum via a
    [128,1]x[128,1] ones-matmul; ctx = sum_t p[:,t] (x) encN-tile chains
    accumulated in PSUM, scaled by 1/sum at the end.  ctx of example b is
    emitted after the score phase of example b+1 so the PE never stalls on
    the softmax tail.
enc is supplied from host in both layouts (encT for scores, encN for
context): 2x DMA (~137 MB/core @ ~360 GB/s) hides fully under the
PE-bound ~550 us runtime.  Measured: rel err ~1e-4 vs fp32 reference,
~0.55 ms/call steady-state on 8 cores.
"""
import numpy as np

import concourse.bass as bass
import concourse.mybir as mybir
import concourse.tile as tile
from concourse import bacc
from concourse.bass import ts
from concourse.bass_utils import run_bass_kernel_spmd

AF = mybir.ActivationFunctionType
ALU = mybir.AluOpType
F32 = mybir.dt.float32
F32R = mybir.dt.float32r

N_CORES = 8
B, S, DK, D = 64, 2048, 1024, 1024
BC = B // N_CORES          # examples per core
KC = DK // 128             # k chunks
MC = D // 128              # m chunks (hidden dim)
NST = S // 128             # s-tiles per example
DH = 2                     # d halves (512 moving cols each)


def build_kernel(reps: int = 1, bc: int = BC, s: int = S, sdt=F32R):
    nst = s // 128
    nc = bacc.Bacc(None)

    encT = nc.dram_tensor("encT", [DK, bc * s], sdt, kind="ExternalInput")
    encN = nc.dram_tensor("encN", [bc * s, DK], F32R, kind="ExternalInput")
    w1eT = nc.dram_tensor("w1eT", [128, KC, D], sdt, kind="ExternalInput")
    w1hT = nc.dram_tensor("w1hT", [128, MC, D], F32R, kind="ExternalInput")
    hT = nc.dram_tensor("hT", [128, MC, bc], F32R, kind="ExternalInput")
    b1r = nc.dram_tensor("b1r", [1, D], F32R, kind="ExternalInput")
    w2r = nc.dram_tensor("w2r", [1, D], F32, kind="ExternalInput")
    out_d = nc.dram_tensor("out", [bc, DK], F32, kind="ExternalOutput")

    with tile.TileContext(nc) as tc:
        with (
            tc.tile_pool(name="consts", bufs=1) as consts,
            tc.tile_pool(name="smalls", bufs=4) as smalls,
            tc.tile_pool(name="prep", bufs=5, space="PSUM") as prep,
            tc.tile_pool(name="miscps", bufs=1, space="PSUM") as miscps,
            tc.tile_pool(name="sumps", bufs=1, space="PSUM") as sumps,
        ):
            # ---- constants / parameters ----
            w1eT_sb = consts.tile([128, KC, D], sdt)
            nc.sync.dma_start(out=w1eT_sb, in_=w1eT[:, :, :])
            # W2 broadcast to all 128 partitions
            w2b_sb = consts.tile([128, D], F32)
            w2_ap = w2r[0:1, :]
            nc.sync.dma_start(
                out=w2b_sb,
                in_=bass.AP(tensor=w2_ap.tensor, offset=w2_ap.offset,
                            ap=[[0, 128]] + list(w2_ap.ap[1:])),
            )
            ones_col = consts.tile([128, 1], F32)
            nc.vector.memset(ones_col, 1.0)
            hb8 = consts.tile([bc, D], F32)

            # ---- startup: hvec = W1h @ h + b1 for all bc examples.
            # W1h lives in a scoped pool released before the streaming pools
            # open (32 KB/partition that the steady state can't afford).
            with tc.tile_pool(name="w1hp", bufs=1) as w1hp:
                w1hT_sb = w1hp.tile([128, MC, D], F32R)
                nc.sync.dma_start(out=w1hT_sb, in_=w1hT[:, :, :])
                hT_sb = w1hp.tile([128, MC, bc], F32R)
                nc.sync.dma_start(out=hT_sb, in_=hT[:, :, :])
                b1b_sb = w1hp.tile([bc, D], F32R)
                b1_ap = b1r[0:1, :]
                nc.sync.dma_start(
                    out=b1b_sb,
                    in_=bass.AP(tensor=b1_ap.tensor, offset=b1_ap.offset,
                                ap=[[0, bc]] + list(b1_ap.ap[1:])),
                )
                hv_ps = miscps.tile([bc, D], F32, tag="misc")
                for mc in range(MC):
                    for dh in range(DH):
                        nc.tensor.matmul(
                            hv_ps[:, ts(dh, 512)],
                            hT_sb[:, mc, :], w1hT_sb[:, mc, ts(dh, 512)],
                            start=(mc == 0), stop=(mc == MC - 1),
                        )
                nc.vector.tensor_add(hb8, hv_ps, b1b_sb)

            with (
                tc.tile_pool(name="hbd_pool", bufs=1, space="DRAM") as hbdp,
                tc.tile_pool(name="encp", bufs=4) as encp,
                tc.tile_pool(name="encn", bufs=10) as encn,
                tc.tile_pool(name="tanhp", bufs=3) as tanhp,
                tc.tile_pool(name="ttrs", bufs=1) as ttrs,
                tc.tile_pool(name="ep", bufs=2) as ep,
                tc.tile_pool(name="hbp", bufs=2) as hbp,
                tc.tile_pool(name="biasp", bufs=3) as biasp,
                tc.tile_pool(name="outp", bufs=2) as outp,
            ):
              hbd = hbdp.tile([bc, D], F32)
              nc.sync.dma_start(out=hbd, in_=hb8[:, :])

              def body(_iv=None):
                # ---- per-example pipeline ----
                def scores_phase(b):
                    # this example's bias row -> partition 0
                    hb_bc = hbp.tile([128, D], F32)
                    hrow_ap = hbd[b:b + 1, :]
                    nc.sync.dma_start(
                        out=hb_bc,
                        in_=bass.AP(tensor=hrow_ap.tensor, offset=hrow_ap.offset,
                                    ap=[[0, 128]] + list(hrow_ap.ap[1:])))
                    e_sb = ep.tile([128, nst], F32, tag="e")
                    p_sb = ep.tile([128, nst], F32R, tag="p")
                    for h in range(s // 512):           # quarters of s
                        encTt = encp.tile([128, KC, 512], sdt)
                        e_ap = encT[:, :]
                        nc.sync.dma_start(
                            out=encTt,
                            in_=bass.AP(tensor=e_ap.tensor,
                                        offset=b * s + h * 512,
                                        ap=[[bc * s, 128], [128 * bc * s, KC],
                                            [1, 512]]),
                        )
                        for t4 in range(4):
                            t = h * 4 + t4
                            tanh_t = tanhp.tile([128, D], F32)
                            for dh in range(DH):
                                pre = prep.tile([128, 512], F32)
                                for kc in range(KC):
                                    nc.tensor.matmul(
                                        pre, encTt[:, kc, ts(t4, 128)],
                                        w1eT_sb[:, kc, ts(dh, 512)],
                                        start=(kc == 0), stop=(kc == KC - 1),
                                    )
                                biased = biasp.tile([128, 512], F32)
                                nc.vector.scalar_tensor_tensor(
                                    out=biased, in0=pre, scalar=0.0,
                                    in1=hb_bc[:, ts(dh, 512)],
                                    op0=ALU.add, op1=ALU.add)
                                nc.scalar.activation(
                                    tanh_t[:, ts(dh, 512)], biased, AF.Tanh)
                            ttr_o = ttrs.tile([128, D], F32, tag="ttr")
                            nc.vector.scalar_tensor_tensor(
                                out=ttr_o, in0=tanh_t, scalar=0.0,
                                in1=w2b_sb, op0=ALU.add, op1=ALU.mult,
                                accum_out=e_sb[:, t:t + 1],
                            )
                    nc.scalar.activation(p_sb, e_sb, AF.Exp)
                    pcs = smalls.tile([128, 1], F32, tag="pcs")
                    nc.vector.reduce_sum(pcs, p_sb, axis=mybir.AxisListType.X)
                    sum_ps = sumps.tile([1, 1], F32)
                    nc.tensor.matmul(sum_ps, pcs, ones_col, start=True, stop=True)
                    rs = smalls.tile([1, 1], F32, tag="rs")
                    nc.vector.reciprocal(rs, sum_ps)
                    return p_sb, rs

                def ctx_phase(b, p_sb, rs):
                    ctx_ps = miscps.tile([1, DK], F32, tag="misc")
                    for t in range(nst):
                        encNt = encn.tile([128, DK], F32R)
                        nc.sync.dma_start(
                            out=encNt,
                            in_=encN[b * s + t * 128: b * s + (t + 1) * 128, :],
                        )
                        for dh in range(DH):
                            nc.tensor.matmul(
                                ctx_ps[:, ts(dh, 512)],
                                p_sb[:, t:t + 1], encNt[:, ts(dh, 512)],
                                start=(t == 0), stop=(t == nst - 1),
                            )
                    ctx_sb = outp.tile([1, DK], F32)
                    nc.vector.tensor_scalar_mul(ctx_sb, ctx_ps, rs)
                    nc.sync.dma_start(out=out_d[b:b + 1, :], in_=ctx_sb)

                prev = None
                for b in range(bc):
                    cur = scores_phase(b)
                    if prev is not None:
                        ctx_phase(b - 1, *prev)
                    prev = cur
                ctx_phase(bc - 1, *prev)

              if reps == 1:
                  body()
              else:
                  with tc.For_i(0, reps, 1) as _i:
                      body(_i)

    nc.compile()
    return nc


def prep_inputs(hidden_state, encoder_outputs, W1, b1, W2, score_np=np.float32):
    """Split + relayout full inputs into per-core in_maps."""
    hidden_state = np.ascontiguousarray(hidden_state, dtype=np.float32)
    encoder_outputs = np.asarray(encoder_outputs, dtype=np.float32)
    W1 = np.asarray(W1, dtype=np.float32)
    b1 = np.asarray(b1, dtype=np.float32)
    W2 = np.asarray(W2, dtype=np.float32)

    W1e, W1h = W1[:, :DK], W1[:, DK:]
    # w1eT[kl, kc, d] = W1e[d, kc*128+kl]
    w1eT = np.ascontiguousarray(W1e.T.reshape(KC, 128, D).transpose(1, 0, 2))
    # w1hT[ml, mc, d] = W1h[d, mc*128+ml]
    w1hT = np.ascontiguousarray(W1h.T.reshape(MC, 128, D).transpose(1, 0, 2))
    b1r = np.ascontiguousarray(b1.reshape(1, D))
    w2r = np.ascontiguousarray(W2.reshape(1, D))

    in_maps = []
    for c in range(N_CORES):
        sl = slice(c * BC, (c + 1) * BC)
        enc_c = encoder_outputs[sl]                      # [BC, S, DK]
        encT = np.ascontiguousarray(
            enc_c.transpose(2, 0, 1).reshape(DK, BC * S))
        encN = np.ascontiguousarray(enc_c.reshape(BC * S, DK))
        h_c = hidden_state[sl]                           # [BC, D]
        hT = np.ascontiguousarray(h_c.T.reshape(MC, 128, BC).transpose(1, 0, 2))
        in_maps.append({
            "encT": encT.astype(score_np), "encN": encN,
            "w1eT": w1eT.astype(score_np), "w1hT": w1hT,
            "hT": hT, "b1r": b1r, "w2r": w2r,
        })
    return in_maps


_NC_CACHE = {}


def kernel(hidden_state, encoder_outputs, W1, b1, W2):
    if "nc" not in _NC_CACHE:
        _NC_CACHE["nc"] = build_kernel(reps=1)
    nc = _NC_CACHE["nc"]
    in_maps = prep_inputs(hidden_state, encoder_outputs, W1, b1, W2)
    res = run_bass_kernel_spmd(nc, in_maps, core_ids=list(range(N_CORES)))
    return np.concatenate([r["out"] for r in res.results], axis=0)



# revision 4
# speedup vs baseline: 1.7384x; 1.7384x over previous
"""Trainium2 Bass kernel for nn_Attention_update (additive attention pooling).

reference math (per example b):
    pre[s,d] = enc[b] @ W1e^T + (W1h @ h[b] + b1)      # [S, D]
    e[s]     = tanh(pre) @ W2[0]                        # [S]
    alpha    = softmax(e);  ctx = alpha @ enc[b]        # [DK]

Sharding: data-parallel over batch B=64 across 8 cores (8 examples/core),
same SPMD program on every core, no collectives.

Per-core kernel:
  - scores matmul in fp8 e4m3 with perf_mode=DoubleRow: each matmul
    contracts TWO 128-k chunks (stationary enc pairs [128,2,128s],
    moving W1e^T pairs [128,2,512d]) -> 2x PE throughput vs f32r.
    W1e is pre-scaled by 32 on host (fp8 dynamic range); the 1/32
    descale rides the PSUM-evacuating bias-add (op0=mult).
    Quantization noise on this data: rel err ~1.1e-2 < 2e-2 gate.
  - bias row (W1h@h+b1, computed once at startup on PE in f32r) is
    broadcast to all 128 partitions via a stride-0 DMA and added on
    VectorE while evacuating PSUM; ScalarE applies tanh.
  - e[s] = sum_d tanh*W2 in one fused VectorE scalar_tensor_tensor with
    accum_out -> e lands directly as [128, 16] (s on partitions).
  - p = exp(e); sum via a [128,1]x[128,1] ones-matmul; ctx = per-s-tile
    p-column (x) encN-tile chains accumulated in PSUM (f32r: alpha needs
    full precision), scaled by 1/sum at the end.  ctx of example b is
    emitted after the score phase of example b+1 so the PE never stalls.
enc is supplied from host in both layouts (encT fp8 for scores, encN f32
for context): ~80 MB/core DMA hides under the PE-bound runtime.
"""
import numpy as np
import ml_dtypes

import concourse.bass as bass
import concourse.mybir as mybir
import concourse.tile as tile
from concourse import bacc
from concourse.bass import ts
from concourse.bass_utils import run_bass_kernel_spmd

AF = mybir.ActivationFunctionType
ALU = mybir.AluOpType
F32 = mybir.dt.float32
F32R = mybir.dt.float32r
FP8 = mybir.dt.float8e4
DR = mybir.MatmulPerfMode.DoubleRow
FP8_NP = ml_dtypes.float8_e4m3

N_CORES = 8
B, S, DK, D = 64, 2048, 1024, 1024
BC = B // N_CORES          # examples per core
KC = DK // 128             # k chunks
KCP = KC // 2              # k chunk pairs (DoubleRow)
MC = D // 128              # m chunks (hidden dim)
NST = S // 128             # s-tiles per example
DH = 2                     # d halves (512 moving cols each)
WSCALE = 32.0              # fp8 pre-scale on W1e


def build_kernel(reps: int = 1, bc: int = BC, s: int = S):
    nst = s // 128
    nc = bacc.Bacc(None)

    encT8 = nc.dram_tensor("encT8", [128, KCP, 2, bc * s], FP8,
                           kind="ExternalInput")
    encN = nc.dram_tensor("encN", [bc * s, DK], F32R, kind="ExternalInput")
    w1e8 = nc.dram_tensor("w1e8", [128, KCP, 2, D], FP8, kind="ExternalInput")
    hvr = nc.dram_tensor("hvr", [bc, D], F32, kind="ExternalInput")
    w2r = nc.dram_tensor("w2r", [1, D], F32, kind="ExternalInput")
    out_d = nc.dram_tensor("out", [bc, DK], F32, kind="ExternalOutput")

    with tile.TileContext(nc) as tc:
        with (
            tc.tile_pool(name="consts", bufs=1) as consts,
            tc.tile_pool(name="smalls", bufs=4) as smalls,
            tc.tile_pool(name="prep", bufs=5, space="PSUM") as prep,
            tc.tile_pool(name="miscps", bufs=1, space="PSUM") as miscps,
            tc.tile_pool(name="sumps", bufs=1, space="PSUM") as sumps,
        ):
            # ---- constants / parameters ----
            w1e8_sb = consts.tile([128, KCP, 2, D], FP8)
            nc.sync.dma_start(out=w1e8_sb, in_=w1e8[:, :, :, :])
            # W2 broadcast to all 128 partitions
            w2b_sb = consts.tile([128, D], F32)
            w2_ap = w2r[0:1, :]
            nc.sync.dma_start(
                out=w2b_sb,
                in_=bass.AP(tensor=w2_ap.tensor, offset=w2_ap.offset,
                            ap=[[0, 128]] + list(w2_ap.ap[1:])),
            )
            ones_col = consts.tile([128, 1], F32)
            nc.vector.memset(ones_col, 1.0)

            with (
                tc.tile_pool(name="encp", bufs=4) as encp,
                tc.tile_pool(name="encn", bufs=10) as encn,
                tc.tile_pool(name="tanhp", bufs=3) as tanhp,
                tc.tile_pool(name="ttrs", bufs=1) as ttrs,
                tc.tile_pool(name="ep", bufs=2) as ep,
                tc.tile_pool(name="hbp", bufs=2) as hbp,
                tc.tile_pool(name="biasp", bufs=3) as biasp,
                tc.tile_pool(name="outp", bufs=2) as outp,
            ):
              def body(_iv=None):
                # ---- per-example pipeline ----
                def scores_phase(b):
                    # this example's bias row -> all 128 partitions
                    hb_bc = hbp.tile([128, D], F32)
                    hrow_ap = hvr[b:b + 1, :]
                    nc.sync.dma_start(
                        out=hb_bc,
                        in_=bass.AP(tensor=hrow_ap.tensor, offset=hrow_ap.offset,
                                    ap=[[0, 128]] + list(hrow_ap.ap[1:])))
                    e_sb = ep.tile([128, nst], F32, tag="e")
                    p_sb = ep.tile([128, nst], F32R, tag="p")
                    for h in range(s // 512):           # quarters of s
                        encTt = encp.tile([128, KCP, 2, 512], FP8)
                        e_ap = encT8[:, :, :, :]
                        nc.sync.dma_start(
                            out=encTt,
                            in_=bass.AP(tensor=e_ap.tensor,
                                        offset=b * s + h * 512,
                                        ap=[[KCP * 2 * bc * s, 128],
                                            [2 * bc * s, KCP],
                                            [bc * s, 2],
                                            [1, 512]]),
                        )
                        for t4 in range(4):
                            t = h * 4 + t4
                            tanh_t = tanhp.tile([128, D], F32)
                            for dh in range(DH):
                                pre = prep.tile([128, 512], F32)
                                for p in range(KCP):
                                    nc.tensor.matmul(
                                        pre, encTt[:, p, :, ts(t4, 128)],
                                        w1e8_sb[:, p, :, ts(dh, 512)],
                                        start=(p == 0), stop=(p == KCP - 1),
                                        perf_mode=DR,
                                    )
                                biased = biasp.tile([128, 512], F32)
                                nc.vector.scalar_tensor_tensor(
                                    out=biased, in0=pre, scalar=1.0 / WSCALE,
                                    in1=hb_bc[:, ts(dh, 512)],
                                    op0=ALU.mult, op1=ALU.add)
                                nc.scalar.activation(
                                    tanh_t[:, ts(dh, 512)], biased, AF.Tanh)
                            ttr_o = ttrs.tile([128, D], F32, tag="ttr")
                            nc.vector.scalar_tensor_tensor(
                                out=ttr_o, in0=tanh_t, scalar=0.0,
                                in1=w2b_sb, op0=ALU.add, op1=ALU.mult,
                                accum_out=e_sb[:, t:t + 1],
                            )
                    nc.scalar.activation(p_sb, e_sb, AF.Exp)
                    pcs = smalls.tile([128, 1], F32, tag="pcs")
                    nc.vector.reduce_sum(pcs, p_sb, axis=mybir.AxisListType.X)
                    sum_ps = sumps.tile([1, 1], F32)
                    nc.tensor.matmul(sum_ps, pcs, ones_col, start=True, stop=True)
                    rs = smalls.tile([1, 1], F32, tag="rs")
                    nc.vector.reciprocal(rs, sum_ps)
                    return p_sb, rs

                def ctx_phase(b, p_sb, rs):
                    ctx_ps = miscps.tile([1, DK], F32, tag="misc")
                    for t in range(nst):
                        encNt = encn.tile([128, DK], F32R)
                        nc.sync.dma_start(
                            out=encNt,
                            in_=encN[b * s + t * 128: b * s + (t + 1) * 128, :],
                        )
                        for dh in range(DH):
                            nc.tensor.matmul(
                                ctx_ps[:, ts(dh, 512)],
                                p_sb[:, t:t + 1], encNt[:, ts(dh, 512)],
                                start=(t == 0), stop=(t == nst - 1),
                            )
                    ctx_sb = outp.tile([1, DK], F32)
                    nc.vector.tensor_scalar_mul(ctx_sb, ctx_ps, rs)
                    nc.sync.dma_start(out=out_d[b:b + 1, :], in_=ctx_sb)

                prev = None
                for b in range(bc):
                    cur = scores_phase(b)
                    if prev is not None:
                        ctx_phase(b - 1, *prev)
                    prev = cur
                ctx_phase(bc - 1, *prev)

              if reps == 1:
                  body()
              else:
                  with tc.For_i(0, reps, 1) as _i:
                      body(_i)

    nc.compile()
    return nc


def prep_inputs(hidden_state, encoder_outputs, W1, b1, W2):
    """Split + relayout full inputs into per-core in_maps."""
    hidden_state = np.ascontiguousarray(hidden_state, dtype=np.float32)
    encoder_outputs = np.asarray(encoder_outputs, dtype=np.float32)
    W1 = np.asarray(W1, dtype=np.float32)
    b1 = np.asarray(b1, dtype=np.float32)
    W2 = np.asarray(W2, dtype=np.float32)

    W1e, W1h = W1[:, :DK], W1[:, DK:]
    # w1e8[kl, kcp, two, d] = W1e[d, (2*kcp+two)*128+kl] * WSCALE, e4m3
    w1e8 = np.ascontiguousarray(
        (W1e.T.reshape(KCP, 2, 128, D) * WSCALE).transpose(2, 0, 1, 3)
    ).astype(FP8_NP)
    # hvec = W1h @ h + b1 on host (134 MFLOP, 0.05% of kernel FLOPs)
    hvec = hidden_state @ W1h.T + b1[None, :]
    w2r = np.ascontiguousarray(W2.reshape(1, D))

    in_maps = []
    for c in range(N_CORES):
        sl = slice(c * BC, (c + 1) * BC)
        enc_c = encoder_outputs[sl]                      # [BC, S, DK]
        # encT8[kl, kcp, two, x] = enc_c.reshape(BC*S, DK)[x, (2*kcp+two)*128+kl]
        encT8 = np.ascontiguousarray(
            enc_c.transpose(2, 0, 1).reshape(KCP, 2, 128, BC * S)
            .transpose(2, 0, 1, 3)).astype(FP8_NP)
        encN = np.ascontiguousarray(enc_c.reshape(BC * S, DK))
        in_maps.append({
            "encT8": encT8, "encN": encN, "w1e8": w1e8,
            "hvr": np.ascontiguousarray(hvec[sl]), "w2r": w2r,
        })
    return in_maps


_NC_CACHE = {}


def kernel(hidden_state, encoder_outputs, W1, b1, W2):
    if "nc" not in _NC_CACHE:
        _NC_CACHE["nc"] = build_kernel(reps=1)
    nc = _NC_CACHE["nc"]
    in_maps = prep_inputs(hidden_state, encoder_outputs, W1, b1, W2)
    res = run_bass_kernel_spmd(nc, in_maps, core_ids=list(range(N_CORES)))
    return np.concatenate([r["out"] for r in res.results], axis=0)


# revision 8
# speedup vs baseline: 2.1655x; 1.2457x over previous
"""Trainium2 Bass kernel for nn_Attention_update (additive attention pooling).

reference math (per example b):
    pre[s,d] = enc[b] @ W1e^T + (W1h @ h[b] + b1)      # [S, D]
    e[s]     = tanh(pre) @ W2[0]                        # [S]
    alpha    = softmax(e);  ctx = alpha @ enc[b]        # [DK]

Sharding: data-parallel over batch B=64 across 8 cores (8 examples/core),
same SPMD program on every core, no collectives.

Per-core kernel:
  - scores matmul in fp8 e4m3 with perf_mode=DoubleRow: each matmul
    contracts TWO 128-k chunks (stationary enc pairs [128,2,128s],
    moving W1e^T pairs [128,2,512d]) -> 2x PE throughput vs f32r.
    W1e is pre-scaled by 32 on host (fp8 dynamic range); the 1/32
    descale rides the tanh activation's scale immediate.
    Quantization noise on this data: rel err ~1.1e-2 < 2e-2 gate.
  - hvec = 32*(W1h@h+b1) precomputed on host (134 MFLOP, 0.05% of total),
    broadcast to 128 partitions by a stride-0 DMA per example; one fused
    VectorE scalar_tensor_tensor adds it over the full [128,1024] PSUM
    tile (2 banks); ScalarE applies tanh(x/32) in one [128,1024] pass,
    emitting bf16.
  - e[s] = sum_d tanh*W2 in one VectorE stt (bf16 inputs -> 2x packed)
    with accum_out -> e lands as [128, 16] (s on partitions).
  - p = exp(e) in bf16; sum via ones-matmul; ctx = per-s-tile p-column
    (x) encN-tile (bf16) accumulated in PSUM over 16 s-tiles, as FOUR
    column-packed concurrent matmul chains (tile_position=(0,32q), d
    quarter q) -> ~3x faster than serial M=1 matmuls.  ctx of example b
    is emitted after the score phase of example b+1 to keep PE busy.
enc is supplied from host in both layouts (encT fp8 for scores, encN bf16
for context): ~48 MB/core DMA hides under the compute.
"""
import numpy as np
import ml_dtypes

import concourse.bass as bass
import concourse.mybir as mybir
import concourse.tile as tile
from concourse import bacc
from concourse.bass import ts
from concourse.bass_utils import run_bass_kernel_spmd

AF = mybir.ActivationFunctionType
ALU = mybir.AluOpType
F32 = mybir.dt.float32
F32R = mybir.dt.float32r
BF16 = mybir.dt.bfloat16
FP8 = mybir.dt.float8e4
DR = mybir.MatmulPerfMode.DoubleRow
FP8_NP = ml_dtypes.float8_e4m3
BF16_NP = ml_dtypes.bfloat16

N_CORES = 8
B, S, DK, D = 64, 2048, 1024, 1024
BC = B // N_CORES          # examples per core
KC = DK // 128             # k chunks
KCP = KC // 2              # k chunk pairs (DoubleRow)
NST = S // 128             # s-tiles per example
DH = 2                     # d halves (512 moving cols each)
DQ = 4                     # d quarters (ctx col-packed chains)
WSCALE = 32.0              # fp8 pre-scale on W1e


def build_kernel(reps: int = 1, bc: int = BC, s: int = S):
    nst = s // 128
    nc = bacc.Bacc(None)

    encT8 = nc.dram_tensor("encT8", [128, KCP, 2, bc * s], FP8,
                           kind="ExternalInput")
    encN = nc.dram_tensor("encN", [bc * s, DK], BF16, kind="ExternalInput")
    w1e8 = nc.dram_tensor("w1e8", [128, KCP, 2, D], FP8, kind="ExternalInput")
    hvr = nc.dram_tensor("hvr", [bc, D], F32, kind="ExternalInput")
    w2r = nc.dram_tensor("w2r", [1, D], BF16, kind="ExternalInput")
    out_d = nc.dram_tensor("out", [bc, DK], F32, kind="ExternalOutput")

    with tile.TileContext(nc) as tc:
        with (
            tc.tile_pool(name="consts", bufs=1) as consts,
            tc.tile_pool(name="smalls", bufs=4) as smalls,
            tc.tile_pool(name="prep", bufs=3, space="PSUM") as prep,
            tc.tile_pool(name="miscps", bufs=1, space="PSUM") as miscps,
            tc.tile_pool(name="sumps", bufs=1, space="PSUM") as sumps,
        ):
            # ---- constants / parameters ----
            w1e8_sb = consts.tile([128, KCP, 2, D], FP8)
            nc.sync.dma_start(out=w1e8_sb, in_=w1e8[:, :, :, :])
            # W2 broadcast to all 128 partitions (bf16)
            w2b_sb = consts.tile([128, D], BF16)
            w2_ap = w2r[0:1, :]
            nc.sync.dma_start(
                out=w2b_sb,
                in_=bass.AP(tensor=w2_ap.tensor, offset=w2_ap.offset,
                            ap=[[0, 128]] + list(w2_ap.ap[1:])),
            )
            ones_col = consts.tile([128, 1], F32)
            nc.vector.memset(ones_col, 1.0)

            with (
                tc.tile_pool(name="encp", bufs=4) as encp,
                tc.tile_pool(name="encn", bufs=10) as encn,
                tc.tile_pool(name="tanhp", bufs=3) as tanhp,
                tc.tile_pool(name="ttrs", bufs=1) as ttrs,
                tc.tile_pool(name="ep", bufs=2) as ep,
                tc.tile_pool(name="hbp", bufs=2) as hbp,
                tc.tile_pool(name="biasp", bufs=3) as biasp,
                tc.tile_pool(name="outp", bufs=2) as outp,
            ):
              def body(_iv=None):
                # ---- per-example pipeline ----
                def scores_phase(b):
                    # this example's bias row (32x scaled) -> all 128 partitions
                    hb_bc = hbp.tile([128, D], F32)
                    hrow_ap = hvr[b:b + 1, :]
                    nc.sync.dma_start(
                        out=hb_bc,
                        in_=bass.AP(tensor=hrow_ap.tensor, offset=hrow_ap.offset,
                                    ap=[[0, 128]] + list(hrow_ap.ap[1:])))
                    e_sb = ep.tile([128, nst], F32, tag="e")
                    p_sb = ep.tile([128, nst], BF16, tag="p")
                    for h in range(s // 512):           # quarters of s
                        encTt = encp.tile([128, KCP, 2, 512], FP8)
                        e_ap = encT8[:, :, :, :]
                        nc.sync.dma_start(
                            out=encTt,
                            in_=bass.AP(tensor=e_ap.tensor,
                                        offset=b * s + h * 512,
                                        ap=[[KCP * 2 * bc * s, 128],
                                            [2 * bc * s, KCP],
                                            [bc * s, 2],
                                            [1, 512]]),
                        )
                        for t4 in range(4):
                            t = h * 4 + t4
                            pre = prep.tile([128, D], F32)
                            for dh in range(DH):
                                for p in range(KCP):
                                    nc.tensor.matmul(
                                        pre[:, ts(dh, 512)],
                                        encTt[:, p, :, ts(t4, 128)],
                                        w1e8_sb[:, p, :, ts(dh, 512)],
                                        start=(p == 0), stop=(p == KCP - 1),
                                        perf_mode=DR,
                                    )
                            biased = biasp.tile([128, D], F32)
                            nc.vector.scalar_tensor_tensor(
                                out=biased, in0=pre, scalar=0.0,
                                in1=hb_bc, op0=ALU.add, op1=ALU.add)
                            tanh_t = tanhp.tile([128, D], BF16)
                            nc.scalar.activation(tanh_t, biased, AF.Tanh,
                                                 scale=1.0 / WSCALE)
                            ttr_o = ttrs.tile([128, D], BF16, tag="ttr")
                            nc.vector.scalar_tensor_tensor(
                                out=ttr_o, in0=tanh_t, scalar=0.0,
                                in1=w2b_sb, op0=ALU.add, op1=ALU.mult,
                                accum_out=e_sb[:, t:t + 1],
                            )
                    nc.scalar.activation(p_sb, e_sb, AF.Exp)
                    pcs = smalls.tile([128, 1], F32, tag="pcs")
                    nc.vector.reduce_sum(pcs, p_sb, axis=mybir.AxisListType.X)
                    sum_ps = sumps.tile([1, 1], F32)
                    nc.tensor.matmul(sum_ps, pcs, ones_col, start=True, stop=True)
                    rs = smalls.tile([1, 1], F32, tag="rs")
                    nc.vector.reciprocal(rs, sum_ps)
                    return p_sb, rs

                def ctx_phase(b, p_sb, rs):
                    # four column-packed concurrent PSUM chains, one per d
                    # quarter, partials landing on partitions {0,32,64,96}
                    ctx_ps = miscps.tile([128, 256], F32, tag="misc")
                    for t in range(nst):
                        encNt = encn.tile([128, DK], BF16)
                        nc.sync.dma_start(
                            out=encNt,
                            in_=encN[b * s + t * 128: b * s + (t + 1) * 128, :],
                        )
                        for q in range(DQ):
                            nc.tensor.matmul(
                                ctx_ps[32 * q:32 * q + 1, :],
                                p_sb[:, t:t + 1], encNt[:, ts(q, 256)],
                                start=(t == 0), stop=(t == nst - 1),
                                tile_position=(0, 32 * q),
                            )
                    ctx_sb = outp.tile([128, 256], F32)
                    for q in range(DQ):
                        nc.vector.tensor_scalar_mul(
                            ctx_sb[32 * q:32 * q + 1, :],
                            ctx_ps[32 * q:32 * q + 1, :], rs)
                    for q in range(DQ):
                        nc.sync.dma_start(
                            out=out_d[b:b + 1, ts(q, 256)],
                            in_=ctx_sb[32 * q:32 * q + 1, :])

                prev = None
                for b in range(bc):
                    cur = scores_phase(b)
                    if prev is not None:
                        ctx_phase(b - 1, *prev)
                    prev = cur
                ctx_phase(bc - 1, *prev)

              if reps == 1:
                  body()
              else:
                  with tc.For_i(0, reps, 1) as _i:
                      body(_i)

    nc.compile()
    return nc


def prep_inputs(hidden_state, encoder_outputs, W1, b1, W2):
    """Split + relayout full inputs into per-core in_maps."""
    hidden_state = np.ascontiguousarray(hidden_state, dtype=np.float32)
    encoder_outputs = np.asarray(encoder_outputs, dtype=np.float32)
    W1 = np.asarray(W1, dtype=np.float32)
    b1 = np.asarray(b1, dtype=np.float32)
    W2 = np.asarray(W2, dtype=np.float32)

    W1e, W1h = W1[:, :DK], W1[:, DK:]
    # w1e8[kl, kcp, two, d] = W1e[d, (2*kcp+two)*128+kl] * WSCALE, e4m3
    w1e8 = np.ascontiguousarray(
        (W1e.T.reshape(KCP, 2, 128, D) * WSCALE).transpose(2, 0, 1, 3)
    ).astype(FP8_NP)
    # hvec = 32*(W1h @ h + b1) on host (134 MFLOP, 0.05% of kernel FLOPs);
    # the 1/32 descale rides the tanh activation scale.
    hvec = (hidden_state @ W1h.T + b1[None, :]) * WSCALE
    w2r = np.ascontiguousarray(W2.reshape(1, D)).astype(BF16_NP)

    in_maps = []
    for c in range(N_CORES):
        sl = slice(c * BC, (c + 1) * BC)
        enc_c = encoder_outputs[sl]                      # [BC, S, DK]
        # encT8[kl, kcp, two, x] = enc_c.reshape(BC*S, DK)[x, (2*kcp+two)*128+kl]
        encT8 = np.ascontiguousarray(
            enc_c.transpose(2, 0, 1).reshape(KCP, 2, 128, BC * S)
            .transpose(2, 0, 1, 3)).astype(FP8_NP)
        encN = np.ascontiguousarray(enc_c.reshape(BC * S, DK)).astype(BF16_NP)
        in_maps.append({
            "encT8": encT8, "encN": encN, "w1e8": w1e8,
            "hvr": np.ascontiguousarray(hvec[sl]), "w2r": w2r,
        })
    return in_maps


_NC_CACHE = {}


def kernel(hidden_state, encoder_outputs, W1, b1, W2):
    if "nc" not in _NC_CACHE:
        _NC_CACHE["nc"] = build_kernel(reps=1)
    nc = _NC_CACHE["nc"]
    in_maps = prep_inputs(hidden_state, encoder_outputs, W1, b1, W2)
    res = run_bass_kernel_spmd(nc, in_maps, core_ids=list(range(N_CORES)))
    return np.concatenate([r["out"] for r in res.results], axis=0)
